# revision 10
# baseline (speedup 1.0000x reference)
"""3x3 conv (256->256, stride 1, pad 1) via 1D Winograd F(2,3) along W,
as implicit GEMM on 8 TRN2 NeuronCores (data-parallel over batch, 4
images/core, weight/bias replicated).

Direct conv needs 9 MACs/output on the PE; F(2,3) along W shares the
kw-reduction between each pair of adjacent output columns: 4 transformed
positions xi, each contracted over (ci, kh) only -> 6 MACs/output, a
1.5x PE-cycle cut (the PE ran at 88% busy / 194 ns per 448-col matmul in
the direct version, so PE cycles are the binding resource).

Per image: DVE computes V[xi] = B^T-combos of x column pairs (4 bf16
adds per (chunk, ci), unit-stride because the host ships x with even-
and odd-logical columns pre-split into separate planes). Per output
row-block (14 rows) and co-half, 4 PSUM banks accumulate
M_xi[co,14h,28c] via 24 matmuls (4 xi x 3 kh x 2 ci, FD=392); vertical
taps clip rows at image edges; the kh=1 (full-coverage) taps run first
so start=True overwrites the whole bank. The output transform
y_even = (M0+M1)+M2, y_odd = (M1-M2)-M3 is 4 DVE fp32 ops writing
interleaved into an SBUF tile (stride-2 via a [...,28,2] view); the
scalar engine adds bias (Identity activation) and issues the store.

Weights ship pre-transformed: U_xi[co,ci,kh] = sum_j G[xi,j] w[...,kh,j]
reordered to [c, xi, kh, ci, co, o] bf16 so each lhsT tile [128,128]
DMAs straight into SBUF. x chunks ride sync (ci=0) / gpsimd (ci=1)
rings as in the direct version; weights+bias+stores on scalar/sync.
"""

from contextlib import ExitStack

import numpy as np
import ml_dtypes

import concourse.bass as bass  # noqa: F401
import concourse.tile as tile
from concourse import bacc, mybir
from concourse.bass_utils import run_bass_kernel_spmd

N_CORES = 8
N_TOTAL = 32
N_PER = N_TOTAL // N_CORES  # 4 images per core
C = 256
H = W = 56
TC = 28         # output column pairs (Winograd tiles along W)
RT = 14         # output rows per block -> PSUM FD = 14*28 = 392 <= 512
NBLK = H // RT  # 4 row blocks
F32 = mybir.dt.float32
BF16 = mybir.dt.bfloat16

# x-chunk row boundaries (chunk c covers rows [CB[c], CB[c+1])); see the
# direct-conv kernel for why 4 chunks (DMA ring semaphore rotation).
CB = [0, 17, 33, 49, 56]
NCH = len(CB) - 1

# per-PSUM-bank matmul order: kh=1 first (full row coverage, so its
# start=True write touches every element), then kh=0, kh=2.
KH_ORDER = [(1, 0), (1, 1), (0, 0), (0, 1), (2, 0), (2, 1)]

# weight DMA pieces, co-split (24 x 65 KB) and ordered by first use
# (group 0 consumes co=0 pieces only; co=1 starts ~24 matmuls later),
# alternating scalar/sync so no piece queues behind a long transfer.
# DMA queue bandwidth is plentiful (~16 engines/queue); arrival order
# within a queue is what matters during image 0's p-state ramp.
W_ORDER = [
    (x, k, co)
    for co in (0, 1)
    for x, k in [
        (0, 1), (0, 0), (0, 2), (1, 1), (1, 0), (1, 2),
        (2, 1), (2, 0), (2, 2), (3, 1), (3, 0), (3, 2),
    ]
]

_CACHE = {}


def _build():
    nc = bacc.Bacc(
        "TRN2", target_bir_lowering=False, debug=False, num_devices=N_CORES
    )
    # x pre-split into odd/even logical columns (o-plane k: col 2k-1,
    # e-plane k: col 2k; 29 data cols padded to 32, zeros elsewhere)
    xs = nc.dram_tensor(
        "xs", [N_PER, 2, 128, H, 2, 32], BF16, kind="ExternalInput"
    ).ap()
    wt = nc.dram_tensor(
        "wt", [128, 4, 3, 2, 2, 128], BF16, kind="ExternalInput"
    ).ap()
    b2 = nc.dram_tensor("b2", [128, 2], F32, kind="ExternalInput").ap()
    out = nc.dram_tensor(
        "out", [N_PER, C, H, W], F32, kind="ExternalOutput"
    ).ap()

    with tile.TileContext(nc) as tc, ExitStack() as ctx:
        wpool = ctx.enter_context(tc.tile_pool(name="w", bufs=1))
        bpool = ctx.enter_context(tc.tile_pool(name="b", bufs=1))
        xpool = ctx.enter_context(tc.tile_pool(name="x", bufs=1))
        vpool = ctx.enter_context(tc.tile_pool(name="v", bufs=1))
        tpool = ctx.enter_context(tc.tile_pool(name="t", bufs=4))
        opool = ctx.enter_context(tc.tile_pool(name="o", bufs=4))
        ppool = ctx.enter_context(tc.tile_pool(name="p", bufs=2, space="PSUM"))

        # persistent double-buffered x and V tiles: [slot][ci]
        x_tiles = [
            [
                xpool.tile(
                    [128, H, 2, 32], BF16, tag=f"x{s}{ci}", name=f"x{s}{ci}"
                )
                for ci in range(2)
            ]
            for s in range(2)
        ]
        v_tiles = [
            [
                vpool.tile(
                    [128, 4, H, TC], BF16, tag=f"v{s}{ci}", name=f"v{s}{ci}"
                )
                for ci in range(2)
            ]
            for s in range(2)
        ]

        # image 0 chunk 0 goes out first on sync/gpsimd so the PE can
        # start ASAP; weight pieces follow on all three rings.
        x0, x1 = x_tiles[0]
        nc.sync.dma_start(x0[:, 0 : CB[1]], xs[0, 0, :, 0 : CB[1]])
        nc.gpsimd.dma_start(x1[:, 0 : CB[1]], xs[0, 1, :, 0 : CB[1]])

        w_sb = wpool.tile([128, 4, 3, 2, 2, 128], BF16)
        for i, (x, k, co) in enumerate(W_ORDER):
            eng = nc.scalar if i % 2 == 0 else nc.sync
            eng.dma_start(
                w_sb[:, x : x + 1, k : k + 1, :, co : co + 1],
                wt[:, x : x + 1, k : k + 1, :, co : co + 1],
            )
        b_sb = bpool.tile([128, 2], F32)
        nc.scalar.dma_start(b_sb[:], b2[:, :])

        for n in range(N_PER):
            s = n % 2
            X = x_tiles[s]
            V = v_tiles[s]
            # x chunk loads: ci=0 on sync, ci=1 on gpsimd
            for c in range(NCH):
                if n == 0 and c == 0:
                    continue
                r0, r1 = CB[c], CB[c + 1]
                nc.sync.dma_start(X[0][:, r0:r1], xs[n, 0, :, r0:r1])
                nc.gpsimd.dma_start(X[1][:, r0:r1], xs[n, 1, :, r0:r1])
            # input transform (DVE, bf16, unit-stride):
            #   v0 = d0-d2 = o[0:28]-o[1:29]   v1 = d1+d2 = e[0:28]+o[1:29]
            #   v2 = d2-d1 = o[1:29]-e[0:28]   v3 = d1-d3 = e[0:28]-e[1:29]
            for c in range(NCH):
                r0, r1 = CB[c], CB[c + 1]
                for ci in range(2):
                    xo0 = X[ci][:, r0:r1, 0, 0:TC]
                    xo1 = X[ci][:, r0:r1, 0, 1 : TC + 1]
                    xe0 = X[ci][:, r0:r1, 1, 0:TC]
                    xe1 = X[ci][:, r0:r1, 1, 1 : TC + 1]
                    nc.vector.tensor_sub(V[ci][:, 0, r0:r1, :], xo0, xo1)
                    nc.vector.tensor_add(V[ci][:, 1, r0:r1, :], xe0, xo1)
                    nc.vector.tensor_sub(V[ci][:, 2, r0:r1, :], xo1, xe0)
                    nc.vector.tensor_sub(V[ci][:, 3, r0:r1, :], xe0, xe1)
            for blk in range(NBLK):
                h0 = blk * RT
                for co in range(2):
                    P = [
                        ppool.tile([128, RT, TC], F32, tag=f"ps{i}", name=f"ps{i}")
                        for i in range(4)
                    ]
                    for x in range(4):
                        for i, (kh, ci) in enumerate(KH_ORDER):
                            dh = kh - 1
                            r0 = max(h0, -dh)
                            r1 = min(h0 + RT, H - dh)
                            nc.tensor.matmul(
                                P[x][:, r0 - h0 : r1 - h0, :],
                                w_sb[:, x, kh, ci, co, :],
                                V[ci][:, x, r0 + dh : r1 + dh, :],
                                start=(i == 0),
                                stop=(i == len(KH_ORDER) - 1),
                            )
                    # output transform: even cols (M0+M1)+M2, odd cols
                    # (M1-M2)-M3, interleaved via the [...,28,2] view.
                    # tensor_tensor may read only ONE input from PSUM
                    # (NCC_IBVF027), so scalar first evicts M1 to SBUF.
                    m1 = tpool.tile([128, RT, TC], F32, tag="m1")
                    nc.scalar.copy(m1, P[1])
                    a = tpool.tile([128, RT, TC], F32, tag="a")
                    b = tpool.tile([128, RT, TC], F32, tag="b")
                    t0 = tpool.tile([128, RT, TC], F32, tag="t0")
                    t1 = tpool.tile([128, RT, TC], F32, tag="t1")
                    nc.vector.tensor_add(a, P[0], m1)
                    nc.vector.tensor_add(t0, a, P[2])
                    nc.vector.tensor_sub(b, m1, P[2])
                    nc.vector.tensor_sub(t1, b, P[3])
                    # scalar interleaves even/odd output columns (stride-2
                    # writes) while adding bias, keeping the store DMA
                    # contiguous
                    o2 = opool.tile([128, RT, TC, 2], F32, tag="o2")
                    nc.scalar.activation(
                        o2[:, :, :, 0],
                        t0,
                        mybir.ActivationFunctionType.Identity,
                        bias=b_sb[:, co : co + 1],
                    )
                    nc.scalar.activation(
                        o2[:, :, :, 1],
                        t1,
                        mybir.ActivationFunctionType.Identity,
                        bias=b_sb[:, co : co + 1],
                    )
                    nc.scalar.dma_start(
                        out[n, co * 128 : (co + 1) * 128, h0 : h0 + RT, :],
                        o2[:],
                    )
    nc.compile()
    return nc


def _get_nc():
    if "nc" not in _CACHE:
        _CACHE["nc"] = _build()
    return _CACHE["nc"]


def _in_maps(x, weight, bias):
    x = np.asarray(x, dtype=np.float32).astype(ml_dtypes.bfloat16)
    weight = np.asarray(weight, dtype=np.float32)
    bias = np.asarray(bias, dtype=np.float32)
    # split x columns: o-plane k = col 2k-1 (k=1..28 -> cols 1,3,..,55),
    # e-plane k = col 2k (k=0..27 -> cols 0,2,..,54); col -1 and 56 are
    # zero padding, cols 29..31 of each plane are zero.
    xr = x.reshape(N_TOTAL, 2, 128, H, W)
    xp = np.zeros((N_TOTAL, 2, 128, H, 2, 32), dtype=ml_dtypes.bfloat16)
    xp[..., 0, 1:29] = xr[..., 1::2]  # odd logical cols 1..55
    xp[..., 1, 0:28] = xr[..., 0::2]  # even logical cols 0..54
    # Winograd-transformed weights:
    # U_x[co,o][ci,c][kh] = sum_j G[x,j] w[co*128+o, ((ci*128+c)*9)+(kh*3+j)]
    # laid out as wt[c, x, kh, ci, co, o] bf16
    G = np.array([[1, 0, 0], [0.5, 0.5, 0.5], [0.5, -0.5, 0.5], [0, 0, 1]],
                 np.float32)
    w6 = weight.reshape(2, 128, 2, 128, 3, 3)  # [co, o, ci, c, kh, kw]
    wt = np.ascontiguousarray(
        np.einsum("xj,AoCckj->cxkCAo", G, w6).astype(ml_dtypes.bfloat16)
    )
    b2 = np.ascontiguousarray(bias.reshape(2, 128).T)
    return [
        {"xs": xp[i * N_PER : (i + 1) * N_PER], "wt": wt, "b2": b2}
        for i in range(N_CORES)
    ]


def _run(x, weight, bias, trace=False):
    res = run_bass_kernel_spmd(
        _get_nc(),
        _in_maps(x, weight, bias),
        core_ids=list(range(N_CORES)),
        trace=trace,
    )
    out = np.concatenate(
        [res.results[i]["out"] for i in range(N_CORES)], axis=0
    )
    return out, res


def kernel(x, weight, bias):
    out, _ = _run(x, weight, bias, trace=False)
    return out


def run_profiled(x, weight, bias):
    out, res = _run(x, weight, bias, trace=True)
    return out, res.exec_time_ns
